# revision 1
# baseline (speedup 1.0000x reference)
"""AnatomicalGCN kernel: pure data-parallel over 8 NeuronCores.

Shards the batch (B=16384) across 8 devices; adjacency + all weights are
replicated. Math matches the reference exactly; matmul operands are cast
to bf16 (f32 accumulation) to halve memory traffic on the big einsums —
well within the 2e-2 relative-error budget.
"""
import functools

import jax
import jax.numpy as jnp
import numpy as np

N_CORES = 8
EPS = 1e-5
N = 12


def _layer_norm(h, g, b):
    m = h.mean(axis=-1, keepdims=True)
    v = ((h - m) ** 2).mean(axis=-1, keepdims=True)
    return (h - m) / jnp.sqrt(v + EPS) * g + b


def _bf16_matmul(h, W):
    # h @ W.T with bf16 inputs, f32 accumulate
    return jax.lax.dot_general(
        h.astype(jnp.bfloat16),
        W.astype(jnp.bfloat16),
        (((h.ndim - 1,), (1,)), ((), ())),
        preferred_element_type=jnp.float32,
    )


def _gat_layer(h, A, W, a):
    Wh = _bf16_matmul(h, W)                                   # (B, N, out)
    d = Wh.shape[-1]
    e_i = Wh @ a[:d]                                          # (B, N)
    e_j = Wh @ a[d:]                                          # (B, N)
    e = jax.nn.leaky_relu(e_i[:, :, None] + e_j[:, None, :], 0.2)
    e = jnp.where(A == 0.0, -jnp.inf, e)
    alpha = jax.nn.softmax(e, axis=-1)
    alpha = jnp.nan_to_num(alpha, nan=0.0)
    alpha = alpha * A
    out = jnp.einsum('bij,bjd->bid', alpha, Wh)
    return jax.nn.elu(out)


def _forward(x, A, W_in, b_in, ln_in_g, ln_in_b, W0, a0, W1, a1, W2, a2,
             ln_g, ln_b):
    h = _bf16_matmul(x, W_in) + b_in
    h = jax.nn.gelu(_layer_norm(h, ln_in_g, ln_in_b), approximate=False)
    for W, a in ((W0, a0), (W1, a1), (W2, a2)):
        h = _gat_layer(h, A, W, a) + h
    node_emb = _layer_norm(h, ln_g, ln_b)
    graph_emb = node_emb.mean(axis=1)
    return node_emb, graph_emb


@functools.partial(jax.pmap, in_axes=(0,) + (None,) * 13)
def _forward_pmap(x, A, W_in, b_in, ln_in_g, ln_in_b, W0, a0, W1, a1, W2,
                  a2, ln_g, ln_b):
    return _forward(x, A, W_in, b_in, ln_in_g, ln_in_b, W0, a0, W1, a1,
                    W2, a2, ln_g, ln_b)


def kernel(x, A, W_in, b_in, ln_in_g, ln_in_b, W0, a0, W1, a1, W2, a2,
           ln_g, ln_b):
    x = np.asarray(x)
    B = x.shape[0]
    xs = x.reshape(N_CORES, B // N_CORES, N, x.shape[-1])
    args = [np.asarray(t) for t in
            (A, W_in, b_in, ln_in_g, ln_in_b, W0, a0, W1, a1, W2, a2,
             ln_g, ln_b)]
    node_emb, graph_emb = _forward_pmap(xs, *args)
    node_emb = np.asarray(node_emb).reshape(B, N, -1)
    graph_emb = np.asarray(graph_emb).reshape(B, -1)
    return node_emb.astype(np.float32), graph_emb.astype(np.float32)
